# revision 1
# baseline (speedup 1.0000x reference)
"""Causal self-attention (GPT-style block) on 8 Trainium2 NeuronCores.

Sharding: tensor-parallel over heads. 16 heads / 8 cores = 2 heads per core.
- c_attn column-parallel: each core computes q/k/v for its 2 heads (128
  channels each of q, k, v) from the full input x.
- attention: fully local per core (its 2 heads, all 4 batches).
- c_proj token-parallel after an on-device AllToAll of the attention
  output (see below); each core returns fully-reduced output rows for its
  own token shard, and the host just concatenates and adds b_proj.

Device kernel notes (all matmuls contract over the partition dim):
- Matmul inputs use float32r (single-pass fp32 on the PE, 4x the fp32 rate;
  ~1.5e-4 input rounding, fp32 accumulate).
- x is fed pre-transposed + tiled from the host: xp[tb, p, kt, s] =
  x[(tb*512+s) token, (kt*128+p) channel] so stage 1 needs no transposes.
- q,k,v are produced channel-major ([chan, token]); v is then PE-transposed
  to token-major tiles with a ones column appended (vaug[.., 64]==1), so a
  single M=65 matmul accumulates both O^T = V^T E and the softmax
  denominator (row 64) per key tile.
- Scores are computed transposed: S^T[key, query] = (k^T).T @ q^T with the
  2 heads packed into the two 64-row halves of the PE array (row tiling).
- Softmax without max-subtraction (logits bounded ~|3| here): E =
  exp(S^T/8) on ACT, causal mask applied multiplicatively on the 4 partial
  (diagonal) key-tiles per query block.
- Normalization: r = 1/l on DVE, broadcast across the 64 head rows with a
  K=1 ones matmul on PE, multiply on DVE. Result lands channel-major in
  yT, which is exactly the stationary layout c_proj needs.
- c_proj is token-parallel: per half-batch (1024 tokens), an on-device
  AllToAll exchanges Y^T slices (each core sends peer j its 2 head-channels
  for peer j's 128 tokens of that half), after which every core holds all
  1024 channels for its own tokens and computes fully-reduced output rows
  with the full w_proj. This cuts per-core PSUM->SBUF eviction and output
  DMA 8x vs row-parallel partial sums. Exchanges are half-batch sized so
  they start mid-batch, and each unit's projection is emitted 3 exchange
  units later so collective latency hides under subsequent attention work
  (engine instruction streams execute in order, so a not-yet-ready
  projection would otherwise stall everything emitted after it).
- The LAST half-batch skips the exchange: waiting out the final
  collective would leave the PE idle with nothing left to overlap, so that
  unit's projection is computed row-parallel (each core: its 128 channels
  x its w_proj row slice -> a partial over all 1024 of its tokens) right
  after the final normalize while the PE is still warm, and the host sums
  those 8 partials for just that token range.
"""

import numpy as np

P = 128
B = 4
T = 2048
BT = B * T            # 8192 tokens
C = 1024
KT = C // P           # 8 contraction tiles of 128 input channels
NTB = BT // 512       # 16 token blocks of 512
HD = 64               # head dim
NQ = T // 512         # 4 query blocks per batch
NCORES = 8

_CACHED = {}


def _build_nc():
    import concourse.mybir as mybir
    import concourse.tile as tile
    from concourse import bacc
    from concourse.masks import make_identity

    f32 = mybir.dt.float32
    f32r = mybir.dt.float32r
    EXP = mybir.ActivationFunctionType.Exp

    nc = bacc.Bacc("TRN2", target_bir_lowering=False, debug=False,
                   num_devices=NCORES)

    TPC = T // NCORES   # 256 tokens per core per batch (proj sharding)

    xp = nc.dram_tensor("xp", [NTB, P, KT, 512], f32r, kind="ExternalInput")
    wq = nc.dram_tensor("wq", [P, KT, P], f32r, kind="ExternalInput")
    wk = nc.dram_tensor("wk", [P, KT, P], f32r, kind="ExternalInput")
    wv = nc.dram_tensor("wv", [P, KT, P], f32r, kind="ExternalInput")
    wp = nc.dram_tensor("wp", [P, KT, C], f32r, kind="ExternalInput")
    bq = nc.dram_tensor("bq", [P, 1], f32, kind="ExternalInput")
    bk = nc.dram_tensor("bk", [P, 1], f32, kind="ExternalInput")
    bv = nc.dram_tensor("bv", [P, 1], f32, kind="ExternalInput")
    wpr = nc.dram_tensor("wpr", [P, C], f32r, kind="ExternalInput")
    yp = nc.dram_tensor("yp", [B, 2, T // 2 // NCORES, C], f32, kind="ExternalOutput")
    ypl = nc.dram_tensor("ypl", [T // 2, C], f32, kind="ExternalOutput")

    with tile.TileContext(nc) as tc:
        with (
            tc.tile_pool(name="const", bufs=1) as const,
            tc.tile_pool(name="xt", bufs=2) as xt_pool,
            tc.tile_pool(name="slab", bufs=2) as slab_pool,
            tc.tile_pool(name="e", bufs=5) as e_pool,
            tc.tile_pool(name="nrm", bufs=2) as nrm_pool,
            tc.tile_pool(name="ob", bufs=3) as ob_pool,
            tc.tile_pool(name="yg", bufs=3) as yg_pool,
            tc.tile_pool(name="dram", bufs=1, space="DRAM") as dram_pool,
            tc.tile_pool(name="ps1", bufs=1, space="PSUM") as ps1_pool,
            tc.tile_pool(name="pss", bufs=2, space="PSUM") as pss_pool,
            tc.tile_pool(name="pso", bufs=2, space="PSUM") as pso_pool,
            tc.tile_pool(name="ppb", bufs=1, space="PSUM") as ppb_pool,
        ):
            TPH = TPC // 2   # 128 tokens per core per half-batch exchange
            g_in = [dram_pool.tile([NCORES, P, TPH], f32r, name=f"g_in{k}",
                                   tag=f"g_in{k}") for k in range(2 * B)]
            g_out = [dram_pool.tile([NCORES, P, TPH], f32r, name=f"g_out{k}",
                                    tag=f"g_out{k}") for k in range(2 * B)]

            # --- constants / weights resident in SBUF ---
            wq_sb = const.tile([P, KT, P], f32r)
            wk_sb = const.tile([P, KT, P], f32r)
            wv_sb = const.tile([P, KT, P], f32r)
            wp_sb = const.tile([P, KT, C], f32r)
            wpr_sb = const.tile([P, C], f32r)
            bq_sb = const.tile([P, 1], f32)
            bk_sb = const.tile([P, 1], f32)
            bv_sb = const.tile([P, 1], f32)
            nc.sync.dma_start(wq_sb[:], wq[:])
            nc.sync.dma_start(bq_sb[:], bq[:])
            nc.sync.dma_start(bk_sb[:], bk[:])
            nc.sync.dma_start(bv_sb[:], bv[:])

            ones_row_f = const.tile([1, HD], f32)
            nc.vector.memset(ones_row_f[:], 1.0)
            ones_row = const.tile([1, HD], f32r)
            nc.vector.tensor_copy(ones_row[:], ones_row_f[:])
            ones_v = const.tile([P, T // P, 1], f32)
            nc.vector.memset(ones_v[:], 1.0)
            ident = const.tile([P, P], f32)
            make_identity(nc, ident[:])

            # mask[p, s] = 1.0 if s >= p else 0.0 (keep upper-right triangle)
            # (built in f32 — gpsimd can't write f32r — then rounded over)
            mask_f = const.tile([P, P], f32)
            nc.gpsimd.memset(mask_f[:], 1.0)
            nc.gpsimd.affine_select(
                out=mask_f[:],
                in_=mask_f[:],
                compare_op=mybir.AluOpType.is_ge,
                fill=0.0,
                base=0,
                pattern=[[1, P]],
                channel_multiplier=-1,
            )
            mask_sb = const.tile([P, P], f32r)
            nc.vector.tensor_copy(mask_sb[:], mask_f[:])

            wp_loaded = []

            def emit_proj(k):
                if not wp_loaded:
                    # deferred so the 4MiB w_proj load doesn't delay the
                    # startup xp streaming
                    nc.sync.dma_start(wp_sb[:], wp[:])
                    nc.sync.dma_start(wpr_sb[:], wpr[:])
                    wp_loaded.append(True)
                # yg[p, cc, t]: channel cc*128+p of my token t of unit k
                yg = yg_pool.tile([P, NCORES, TPH], f32r, tag="yg")
                nc.sync.dma_start(yg[:], g_out[k].rearrange("c p t -> p c t"))
                pp0 = ppb_pool.tile([P, 512], f32, tag="ppb")
                pp1 = ppb_pool.tile([P, 512], f32, tag="ppb")
                for ct in range(KT):
                    nc.tensor.matmul(pp0[:], yg[:, ct, :], wp_sb[:, ct, 0:512],
                                     start=(ct == 0), stop=(ct == KT - 1))
                for ct in range(KT):
                    nc.tensor.matmul(pp1[:], yg[:, ct, :], wp_sb[:, ct, 512:C],
                                     start=(ct == 0), stop=(ct == KT - 1))
                ob = ob_pool.tile([P, C], f32, tag="ob")
                nc.vector.tensor_copy(ob[:, 0:512], pp0[:])
                nc.scalar.copy(ob[:, 512:C], pp1[:])
                nc.sync.dma_start(yp[k // 2, k % 2, :, :], ob[:])

            def emit_proj_partial(yTh):
                # ping-pong between two psum pools so the tail chain is
                # paced by DVE evictions, not psum-slot round trips
                for tt in range(T // 2 // P):
                    tsl = slice(tt * P, (tt + 1) * P)
                    pp0 = ppb_pool.tile([P, 512], f32, tag="ppb")
                    pp1 = ps1_pool.tile([P, 512], f32, tag="ps1")
                    nc.tensor.matmul(pp0[:], yTh[:, tsl], wpr_sb[:, 0:512],
                                     start=True, stop=True)
                    nc.tensor.matmul(pp1[:], yTh[:, tsl], wpr_sb[:, 512:C],
                                     start=True, stop=True)
                    ob = ob_pool.tile([P, C], f32, tag="ob")
                    nc.vector.tensor_copy(ob[:, 0:512], pp0[:])
                    nc.scalar.copy(ob[:, 512:C], pp1[:])
                    nc.sync.dma_start(ypl[tsl, :], ob[:])

            def emit_exchange(k, yTh):
                # peer j gets my 2 head-channels for its 128 tokens of unit k
                for j in range(NCORES):
                    nc.sync.dma_start(g_in[k][j], yTh[:, j * TPH:(j + 1) * TPH])
                nc.gpsimd.collective_compute(
                    "AllToAll",
                    mybir.AluOpType.bypass,
                    replica_groups=[list(range(NCORES))],
                    ins=[g_in[k][:]],
                    outs=[g_out[k][:]],
                )

            for b in range(B):
                # --- stage 1: q^T, k^T, v^T (channel-major, f32r) ---
                qT = slab_pool.tile([P, T], f32r, tag="qT")
                kT = slab_pool.tile([P, T], f32r, tag="kT")
                vT = slab_pool.tile([P, T], f32, tag="scratch")
                # token-major v with ones cols at 64 (h0) and 129 (h1)
                vaug = slab_pool.tile([P, T // P, 2 * HD + 2], f32r, tag="vaug")
                nc.vector.tensor_copy(vaug[:, :, HD:HD + 1], ones_v[:])
                nc.vector.tensor_copy(vaug[:, :, 2 * HD + 1:2 * HD + 2], ones_v[:])

                for lb in range(NQ):
                    tb = b * NQ + lb
                    xt = xt_pool.tile([P, KT, 512], f32r)
                    if tb == 0:
                        # first block on the gpsimd queue, parallel to the
                        # weight loads on the sync queue
                        nc.gpsimd.dma_start(xt[:], xp[tb])
                    else:
                        nc.sync.dma_start(xt[:], xp[tb])
                    if tb == 0:
                        # behind the first x block: k/v weights aren't needed
                        # until after the first q matmul group
                        nc.sync.dma_start(wk_sb[:], wk[:])
                        nc.sync.dma_start(wv_sb[:], wv[:])
                    sl = slice(lb * 512, (lb + 1) * 512)

                    for w_sb, b_sb, dst in ((wq_sb, bq_sb, qT),
                                            (wk_sb, bk_sb, kT),
                                            (wv_sb, bv_sb, vT)):
                        ps = ps1_pool.tile([P, 512], f32, tag="ps1")
                        for kt in range(KT):
                            nc.tensor.matmul(ps[:], w_sb[:, kt, :], xt[:, kt, :],
                                             start=(kt == 0), stop=(kt == KT - 1))
                        nc.vector.tensor_scalar_add(dst[:, sl], ps[:], b_sb[:])

                    # transpose v to token-major [tok, chan] tiles
                    for t4 in range(4):
                        j = lb * 4 + t4
                        pst = ps1_pool.tile([P, P], f32, tag="ps1")
                        nc.tensor.transpose(pst[:], vT[:, j * P:(j + 1) * P], ident[:])
                        nc.vector.tensor_copy(vaug[:, j, 0:HD], pst[:, 0:HD])
                        nc.vector.tensor_copy(vaug[:, j, HD + 1:2 * HD + 1],
                                              pst[:, HD:P])

                # --- stage 2: attention, per query block ---
                for i in range(NQ):
                    if i % 2 == 0:
                        yT = slab_pool.tile([P, T // 2], f32r, tag="scratch",
                                            name=f"yT_{b}_{i // 2}")
                    isl = slice((i % 2) * 512, (i % 2 + 1) * 512)
                    nj = 4 * (i + 1)
                    po0 = pso_pool.tile([P, 512], f32, tag="pso")
                    po1 = pso_pool.tile([P, 512], f32, tag="pso")

                    def ranges(j):
                        # diagonal tiles: queries below q0 can't see this key
                        # tile — compute only the [q0, 512) query range
                        q0 = max(0, j - 4 * i) * P
                        return q0, slice(q0, 512), slice(512 + q0, 1024)

                    def emit_s(j):
                        # both heads' scores side by side in one 2-bank
                        # psum tile -> a single exp per key tile
                        q0, vsl, v1 = ranges(j)
                        jsl = slice(j * P, (j + 1) * P)
                        qsl = slice(i * 512 + q0, (i + 1) * 512)
                        psp = pss_pool.tile([P, 1024], f32, tag="pss",
                                            name=f"psp{j % 2}")
                        nc.tensor.matmul(psp[:, vsl], kT[0:HD, jsl], qT[0:HD, qsl],
                                         start=True, stop=True, tile_position=(0, 0))
                        nc.tensor.matmul(psp[:, v1], kT[HD:P, jsl], qT[HD:P, qsl],
                                         start=True, stop=True, tile_position=(HD, 0))
                        ep = e_pool.tile([P, 1024], f32r, tag="e", name=f"ep{j % 2}")
                        if q0 == 0:
                            nc.scalar.activation(ep[:], psp[:], EXP, scale=0.125)
                        else:
                            nc.scalar.activation(ep[:, vsl], psp[:, vsl], EXP,
                                                 scale=0.125)
                            nc.scalar.activation(ep[:, v1], psp[:, v1], EXP,
                                                 scale=0.125)
                        if j - 4 * i >= 0:
                            for q in (q0, 512 + q0):
                                msl = slice(q, q + P)
                                nc.vector.tensor_mul(ep[:, msl], ep[:, msl],
                                                     mask_sb[:])
                        return ep

                    # software-pipelined: S(j+1) is issued before PV(j) so the
                    # PE never sits directly behind ACT's exp latency
                    eps = {0: emit_s(0)}
                    for j in range(nj):
                        if j + 1 < nj:
                            eps[j + 1] = emit_s(j + 1)
                        ep = eps.pop(j)
                        q0, vsl, v1 = ranges(j)
                        st, sp = (j == 0), (j == nj - 1)
                        nc.tensor.matmul(po0[0:HD + 1, vsl], vaug[:, j, 0:HD + 1],
                                         ep[:, vsl], start=st, stop=sp)
                        nc.tensor.matmul(po1[0:HD + 1, vsl],
                                         vaug[:, j, HD + 1:2 * HD + 2], ep[:, v1],
                                         start=st, stop=sp)

                    # normalize: yT[head, isl] = O^T * (1/l) broadcast over rows
                    with nc.allow_low_precision(reason="f32r matmul inputs"):
                        r0 = nrm_pool.tile([1, 512], f32r, tag="r")
                        r1 = nrm_pool.tile([1, 512], f32r, tag="r")
                        nc.vector.reciprocal(r0[:], po0[HD:HD + 1, :])
                        nc.vector.reciprocal(r1[:], po1[HD:HD + 1, :])
                    pb0 = ppb_pool.tile([P, 512], f32, tag="ppb")
                    pb1 = ppb_pool.tile([P, 512], f32, tag="ppb")
                    nc.tensor.matmul(pb0[0:HD, :], ones_row[:], r0[:],
                                     start=True, stop=True)
                    nc.tensor.matmul(pb1[0:HD, :], ones_row[:], r1[:],
                                     start=True, stop=True)
                    rb0 = nrm_pool.tile([HD, 512], f32, tag="rb")
                    rb1 = nrm_pool.tile([HD, 512], f32, tag="rb")
                    nc.vector.tensor_copy(rb0[:], pb0[0:HD, :])
                    nc.vector.tensor_copy(rb1[:], pb1[0:HD, :])
                    nc.vector.tensor_mul(yT[0:HD, isl], po0[0:HD, :], rb0[:])
                    nc.vector.tensor_mul(yT[HD:P, isl], po1[0:HD, :], rb1[:])

                    if i % 2 == 1:
                        k = 2 * b + i // 2
                        if k < 2 * B - 1:
                            emit_exchange(k, yT)
                            if k >= 3:
                                emit_proj(k - 3)
                        else:
                            # tail: the three ready exchange-projections go
                            # first (PE-dense), the DVE-paced partial drains
                            # behind them
                            for kk in range(2 * B - 4, 2 * B - 1):
                                emit_proj(kk)
                            emit_proj_partial(yT)

    nc.compile()
    return nc


def _prep_inputs(x, w_attn, b_attn, w_proj):
    x = np.asarray(x, dtype=np.float32)
    w_attn = np.asarray(w_attn, dtype=np.float32)
    b_attn = np.asarray(b_attn, dtype=np.float32)
    w_proj = np.asarray(w_proj, dtype=np.float32)

    x_flat = x.reshape(BT, C)
    # xp[tb, p, kt, s] = x_flat[tb*512+s, kt*128+p]
    xp = np.ascontiguousarray(
        x_flat.T.reshape(KT, P, NTB, 512).transpose(2, 1, 0, 3))

    wp = np.ascontiguousarray(w_proj.reshape(KT, P, C).transpose(1, 0, 2))
    in_maps = []
    for c in range(NCORES):
        cols = slice(P * c, P * (c + 1))

        def wslice(off):
            w = w_attn[:, off + P * c: off + P * (c + 1)]   # [1024, 128]
            return np.ascontiguousarray(w.reshape(KT, P, P).transpose(1, 0, 2))

        in_maps.append({
            "xp": xp,
            "wq": wslice(0),
            "wk": wslice(C),
            "wv": wslice(2 * C),
            "wp": wp,
            "wpr": np.ascontiguousarray(w_proj[cols, :]),
            "bq": np.ascontiguousarray(b_attn[cols]).reshape(P, 1),
            "bk": np.ascontiguousarray(b_attn[C + P * c: C + P * (c + 1)]).reshape(P, 1),
            "bv": np.ascontiguousarray(b_attn[2 * C + P * c: 2 * C + P * (c + 1)]).reshape(P, 1),
        })
    return in_maps


def kernel(x, w_attn, b_attn, w_proj, b_proj):
    from concourse.bass_utils import run_bass_kernel_spmd

    if "nc" not in _CACHED:
        _CACHED["nc"] = _build_nc()
    nc = _CACHED["nc"]

    in_maps = _prep_inputs(x, w_attn, b_attn, w_proj)
    res = run_bass_kernel_spmd(nc, in_maps, core_ids=list(range(NCORES)))

    # core c holds tokens [h*1024 + c*128, +128) of each batch half h,
    # except the last half-batch which comes back as row-parallel partials
    y = np.empty((B, T, C), dtype=np.float32)
    for c in range(NCORES):
        part = res.results[c]["yp"]          # [B, 2, 128, C]
        for h in range(2):
            y[:, h * (T // 2) + c * 128: h * (T // 2) + (c + 1) * 128, :] = part[:, h]
    acc = res.results[0]["ypl"].astype(np.float32).copy()
    for c in range(1, NCORES):
        acc += res.results[c]["ypl"]
    y[B - 1, T // 2:, :] = acc
    y += np.asarray(b_proj, dtype=np.float32)
    return y



# revision 7
# speedup vs baseline: 1.0707x; 1.0707x over previous
"""Causal self-attention (GPT-style block) on 8 Trainium2 NeuronCores.

Sharding: tensor-parallel over heads (16 heads / 8 cores = 2 heads/core).
c_attn column-parallel from full x; attention fully local per core;
c_proj token-parallel after on-device AllToAll exchanges for batches
0..2, row-parallel for batch 3 (host sums the 8 partials).

Key structural choices (all matmuls contract over the partition dim;
matmul inputs are bf16 so every matmul runs at 1 cycle/row on the PE
regardless of free size):
- x is host-pretransposed/tiled to xp[tb, p, kt, s] (bf16) so stage 1
  needs no device transposes.
- q,k produced channel-major ([chan, tok]); v produced DIRECTLY
  token-major (stationary = x tile, moving = w_v) into vaug tiles
  [tok, v_h0(64) | 1 | v_h1(64) | 1]; the ones columns accumulate the
  softmax denominators inside the PV matmuls.
- b_q/b_k folded into the psum eviction (tensor_scalar_add); b_v is
  folded into b_proj ON THE HOST (b_v @ w_proj is a constant row).
- Scores computed transposed per 512-query block: S^T[key, q] with the
  2 heads packed in one [P, 2, 512] psum tile (row-tiled PE loads);
  exp is a single ACT instruction per key tile (strided AP covers both
  heads, including partial diagonal widths); causal mask applied
  multiplicatively to E on DVE.
- PV is TOKEN-major: out[q, chan] = sum_k E^T[k,q] v[k,chan] with
  stationary = 128x128 E tile, moving = vaug slice (65 free). K=128
  and out partitions = queries: half the PE cost of channel-major PV,
  and softmax normalization becomes a per-partition tensor_scalar_mul
  fused into the eviction. A PE transpose restores channel-major yT
  for the projection.
- c_proj: one AllToAll per batch 0..2 (the collective cost model has
  ~15us fixed overhead, so fewer+bigger wins); batch b's projection is
  emitted late in attention(b+1), hiding the collective latency.
- Stage 1 of batch b+1, ready projections, and b3's row-parallel
  partial projections are interleaved into attention's instruction
  stream via filler queues, so the PE never idles behind ACT's exp
  latency.
"""

import collections

import numpy as np

P = 128
B = 4
T = 2048
BT = B * T            # 8192 tokens
C = 1024
KT = C // P           # 8 contraction tiles of 128 input channels
NTB = BT // 512       # 16 token blocks of 512
HD = 64               # head dim
NQ = T // 512         # 4 query blocks per batch
NCORES = 8
TPC = 256             # tokens per core per exchanged batch

_CACHED = {}


def _build_nc():
    import concourse.mybir as mybir
    import concourse.tile as tile
    from concourse import bacc
    from concourse.masks import make_identity

    f32 = mybir.dt.float32
    bf16 = mybir.dt.bfloat16
    EXP = mybir.ActivationFunctionType.Exp

    nc = bacc.Bacc("TRN2", target_bir_lowering=False, debug=False,
                   num_devices=NCORES)

    xp = nc.dram_tensor("xp", [NTB, P, KT, 512], bf16, kind="ExternalInput")
    wq = nc.dram_tensor("wq", [P, KT, P], bf16, kind="ExternalInput")
    wk = nc.dram_tensor("wk", [P, KT, P], bf16, kind="ExternalInput")
    wv = nc.dram_tensor("wv", [P, KT, P], bf16, kind="ExternalInput")
    wp = nc.dram_tensor("wp", [P, KT, C], bf16, kind="ExternalInput")
    wpr = nc.dram_tensor("wpr", [P, C], bf16, kind="ExternalInput")
    bq = nc.dram_tensor("bq", [P, 1], f32, kind="ExternalInput")
    bk = nc.dram_tensor("bk", [P, 1], f32, kind="ExternalInput")
    # outputs: ypx[u] = batch u, this core's 256 tokens; ypl = batch 3
    # row-parallel partial over all 2048 tokens (host sums 8 cores)
    ypx = nc.dram_tensor("ypx", [3, TPC, C], f32, kind="ExternalOutput")
    ypl = nc.dram_tensor("ypl", [T, C], f32, kind="ExternalOutput")

    with tile.TileContext(nc) as tc:
        with (
            tc.tile_pool(name="const", bufs=1) as const,
            tc.tile_pool(name="xt", bufs=2) as xt_pool,
            tc.tile_pool(name="slab", bufs=2) as slab_pool,
            tc.tile_pool(name="e", bufs=3) as e_pool,
            tc.tile_pool(name="ytn", bufs=3) as ytn_pool,
            tc.tile_pool(name="nrm", bufs=4) as nrm_pool,
            tc.tile_pool(name="ob", bufs=3) as ob_pool,
            tc.tile_pool(name="yg", bufs=2) as yg_pool,
            tc.tile_pool(name="dram", bufs=1, space="DRAM") as dram_pool,
            tc.tile_pool(name="pss", bufs=2, space="PSUM") as pss_pool,
            tc.tile_pool(name="pvo", bufs=2, space="PSUM") as pvo_pool,
            tc.tile_pool(name="ps1", bufs=2, space="PSUM") as ps1_pool,
        ):
            g_in = [dram_pool.tile([NCORES, P, TPC], bf16,
                                   name=f"g_in{u}", tag=f"g_in{u}")
                    for u in range(3)]
            g_out = [dram_pool.tile([NCORES, P, TPC], bf16,
                                    name=f"g_out{u}", tag=f"g_out{u}")
                     for u in range(3)]

            # --- constants / weights resident in SBUF ---
            wq_sb = const.tile([P, KT, P], bf16)
            wk_sb = const.tile([P, KT, P], bf16)
            wv_sb = const.tile([P, KT, P], bf16)
            wp_sb = const.tile([P, KT, C], bf16)
            wpr_sb = const.tile([P, C], bf16)
            bq_sb = const.tile([P, 1], f32)
            bk_sb = const.tile([P, 1], f32)

            ident_f = const.tile([P, P], f32)
            make_identity(nc, ident_f[:])
            ident = const.tile([P, P], bf16)
            nc.vector.tensor_copy(ident[:], ident_f[:])

            # mask[p, s] = 1.0 if s >= p else 0.0 (keep q >= k)
            mask_f = const.tile([P, P], f32)
            nc.gpsimd.memset(mask_f[:], 1.0)
            nc.gpsimd.affine_select(
                out=mask_f[:],
                in_=mask_f[:],
                compare_op=mybir.AluOpType.is_ge,
                fill=0.0,
                base=0,
                pattern=[[1, P]],
                channel_multiplier=-1,
            )
            mask2 = const.tile([P, 2, P], bf16)
            nc.vector.tensor_copy(mask2[:, 0, :], mask_f[:])
            nc.vector.tensor_copy(mask2[:, 1, :], mask_f[:])

            nc.sync.dma_start(bq_sb[:], bq[:])
            nc.sync.dma_start(bk_sb[:], bk[:])
            nc.sync.dma_start(wq_sb[:], wq[:])
            # w_proj loads deferred off the startup path; scalar queue so
            # they don't delay xp streaming on the sync queue
            wp_loaded = []

            def load_wp():
                if not wp_loaded:
                    nc.scalar.dma_start(wp_sb[:], wp[:])
                    nc.scalar.dma_start(wpr_sb[:], wpr[:])
                    wp_loaded.append(True)

            slabs = {}
            yTs = {}

            # ---------------- stage 1 (qkv) ----------------
            def stage1_units(b, startup=False):
                """Emission closures for batch b's qkv work, ordered so each
                x-block's DMA is issued ~4 units before its first use."""
                qT = slab_pool.tile([P, T], bf16, tag="qT", name=f"qT{b}")
                kT = slab_pool.tile([P, T], bf16, tag="kT", name=f"kT{b}")
                vaug = slab_pool.tile([P, T // P, 2 * HD + 2], bf16,
                                      tag="vaug", name=f"vaug{b}")
                slabs[b] = (qT, kT, vaug)

                def ones_cols():
                    nc.vector.memset(vaug[:, :, HD:HD + 1], 1.0)
                    nc.vector.memset(vaug[:, :, 2 * HD + 1:2 * HD + 2], 1.0)

                xts = {}

                def load_xt(lb, chunked=False):
                    def emit():
                        xt = xt_pool.tile([P, KT, 512], bf16, tag="xt",
                                          name=f"xt{b}_{lb}")
                        xts[lb] = xt
                        tb = b * NQ + lb
                        if chunked:
                            # per-kt chunks so the first matmul can start
                            # ~8x earlier; k/v weights slotted between
                            for kt in range(KT):
                                nc.sync.dma_start(xt[:, kt, :],
                                                  xp[tb, :, kt, :])
                                if kt == 3:
                                    nc.sync.dma_start(wk_sb[:], wk[:])
                                elif kt == 5:
                                    nc.sync.dma_start(wv_sb[:], wv[:])
                        else:
                            nc.sync.dma_start(xt[:], xp[tb])
                    return emit

                def qk_group(lb, w_sb, b_sb, dst):
                    def emit():
                        xt = xts[lb]
                        sl = slice(lb * 512, (lb + 1) * 512)
                        ps = ps1_pool.tile([P, 512], f32, tag="ps1",
                                           name=f"ps_qk{b}_{lb}")
                        for kt in range(KT):
                            nc.tensor.matmul(ps[:], w_sb[:, kt, :],
                                             xt[:, kt, :],
                                             start=(kt == 0),
                                             stop=(kt == KT - 1))
                        nc.vector.tensor_scalar_add(dst[:, sl], ps[:], b_sb[:])
                    return emit

                def v_group(lb, pair):
                    def emit():
                        xt = xts[lb]
                        psv = ps1_pool.tile([P, 512], f32, tag="ps1",
                                            name=f"ps_v{b}_{lb}_{pair}")
                        for t4 in (2 * pair, 2 * pair + 1):
                            off = t4 * P
                            tsl = slice(off, off + P)
                            for kt in range(KT):
                                nc.tensor.matmul(psv[:, tsl],
                                                 xt[:, kt, tsl],
                                                 wv_sb[:, kt, :],
                                                 start=(kt == 0),
                                                 stop=(kt == KT - 1))
                            j4 = lb * 4 + t4
                            nc.vector.tensor_copy(vaug[:, j4, 0:HD],
                                                  psv[:, off:off + HD])
                            nc.vector.tensor_copy(
                                vaug[:, j4, HD + 1:2 * HD + 1],
                                psv[:, off + HD:off + P])
                    return emit

                units = [ones_cols,
                         load_xt(0, chunked=startup),
                         qk_group(0, wq_sb, bq_sb, qT),
                         load_xt(1),
                         qk_group(0, wk_sb, bk_sb, kT),
                         v_group(0, 0), v_group(0, 1)]
                for lb in range(1, NQ):
                    units.append(qk_group(lb, wq_sb, bq_sb, qT))
                    if lb + 1 < NQ:
                        units.append(load_xt(lb + 1))
                    units.append(qk_group(lb, wk_sb, bk_sb, kT))
                    units.append(v_group(lb, 0))
                    units.append(v_group(lb, 1))
                return units

            # ---------------- c_proj pieces ----------------
            def proj_unit(u, tok0):
                """One 128-token projection tile from exchanged batch u."""
                def emit():
                    yg = yg_tiles[u]
                    tsl = slice(tok0, tok0 + P)
                    pp0 = ps1_pool.tile([P, 512], f32, tag="ps1",
                                        name=f"pp0_{u}_{tok0}")
                    pp1 = ps1_pool.tile([P, 512], f32, tag="ps1",
                                        name=f"pp1_{u}_{tok0}")
                    for ct in range(KT):
                        nc.tensor.matmul(pp0[:], yg[:, ct, tsl],
                                         wp_sb[:, ct, 0:512],
                                         start=(ct == 0), stop=(ct == KT - 1))
                    for ct in range(KT):
                        nc.tensor.matmul(pp1[:], yg[:, ct, tsl],
                                         wp_sb[:, ct, 512:C],
                                         start=(ct == 0), stop=(ct == KT - 1))
                    ob = ob_pool.tile([P, C], f32, tag="ob",
                                      name=f"ob_{u}_{tok0}")
                    nc.vector.tensor_copy(ob[:, 0:512], pp0[:])
                    nc.scalar.copy(ob[:, 512:C], pp1[:])
                    nc.scalar.dma_start(ypx[u, tsl, :], ob[:])
                return emit

            def partial_unit(pt):
                """Row-parallel partial proj for b3 tokens [128pt, +128)."""
                def emit():
                    yT3 = yTs[3]
                    ssl = slice(pt * P, (pt + 1) * P)
                    pp0 = ps1_pool.tile([P, 512], f32, tag="ps1",
                                        name=f"ppl0_{pt}")
                    pp1 = ps1_pool.tile([P, 512], f32, tag="ps1",
                                        name=f"ppl1_{pt}")
                    nc.tensor.matmul(pp0[:], yT3[:, ssl], wpr_sb[:, 0:512],
                                     start=True, stop=True)
                    nc.tensor.matmul(pp1[:], yT3[:, ssl], wpr_sb[:, 512:C],
                                     start=True, stop=True)
                    ob = ob_pool.tile([P, C], f32, tag="ob", name=f"obl_{pt}")
                    nc.vector.tensor_copy(ob[:, 0:512], pp0[:])
                    nc.scalar.copy(ob[:, 512:C], pp1[:])
                    nc.scalar.dma_start(ypl[ssl, :], ob[:])
                return emit

            yg_tiles = {}

            def emit_exchange(u):
                """AllToAll batch u's yT; peer j gets this core's 2
                head-channels for peer j's 256 tokens."""
                yT_ = yTs[u]
                for j in range(NCORES):
                    nc.sync.dma_start(g_in[u][j],
                                      yT_[:, j * TPC:(j + 1) * TPC])
                nc.gpsimd.collective_compute(
                    "AllToAll",
                    mybir.AluOpType.bypass,
                    replica_groups=[list(range(NCORES))],
                    ins=[g_in[u][:]],
                    outs=[g_out[u][:]],
                )
                # gather on the gpsimd queue: it naturally orders after the
                # collective without blocking any busy engine's queue
                yg = yg_pool.tile([P, NCORES, TPC], bf16, tag="yg",
                                  name=f"yg{u}")
                nc.gpsimd.dma_start(yg[:], g_out[u].rearrange("c p t -> p c t"))
                yg_tiles[u] = yg

            # ---------------- attention ----------------
            def attention(b, fillers, late2=(), late3=()):
                qT, kT, vaug = slabs[b]
                yT = slab_pool.tile([P, T], bf16, tag="yT", name=f"yT{b}")
                yTs[b] = yT
                deferred = collections.deque()  # norm->transpose closures
                late2 = collections.deque(late2)
                late3 = collections.deque(late3)

                import os
                DBG_NOFILL = bool(os.environ.get("DBG_NOFILL"))

                def pump(i, j_done, j_total):
                    while deferred:
                        deferred.popleft()()
                    if DBG_NOFILL:
                        return
                    if i >= 2 and late2:
                        late2.popleft()()
                        return
                    if i >= 3 and late3:
                        late3.popleft()()
                        return
                    rem_j = j_total - j_done
                    while fillers and len(fillers) >= rem_j:
                        fillers.popleft()()
                    if fillers and (j_done % 2 == 0):
                        fillers.popleft()()

                j_total = sum(4 * (i + 1) for i in range(NQ))
                j_done = 0
                for i in range(NQ):
                    nj = 4 * (i + 1)
                    pvo_a = pvo_pool.tile([P, 512], f32, tag="pvo",
                                          name=f"pvo_a{b}_{i}")
                    pvo_b = pvo_pool.tile([P, 512], f32, tag="pvo",
                                          name=f"pvo_b{b}_{i}")
                    pvs = (pvo_a, pvo_b)
                    # 8 accumulation regions share 2 banks: a start flag
                    # would lazy-zero the whole 2KB zero region and wipe
                    # sibling partials, so pre-zero and accumulate-only
                    nc.vector.memset(pvo_a[:], 0.0)
                    nc.vector.memset(pvo_b[:], 0.0)

                    def emit_s(j):
                        q0 = max(0, (j - 4 * i)) * P
                        jsl = slice(j * P, (j + 1) * P)
                        qsl = slice(i * 512 + q0, (i + 1) * 512)
                        psp = pss_pool.tile([P, 2, 512], f32, tag="pss",
                                            name=f"psp{b}_{i}_{j}")
                        nc.tensor.matmul(psp[:, 0, q0:512], kT[0:HD, jsl],
                                         qT[0:HD, qsl], start=True, stop=True,
                                         tile_position=(0, 0))
                        nc.tensor.matmul(psp[:, 1, q0:512], kT[HD:P, jsl],
                                         qT[HD:P, qsl], start=True, stop=True,
                                         tile_position=(HD, 0))
                        return psp

                    def emit_e(j, psp):
                        q0 = max(0, (j - 4 * i)) * P
                        ep = e_pool.tile([P, 2, 512], bf16, tag="e",
                                         name=f"ep{b}_{i}_{j}")
                        if q0 == 0:
                            nc.scalar.activation(ep[:, :, :], psp[:, :, :],
                                                 EXP, scale=0.125)
                        else:
                            nc.scalar.activation(ep[:, 0, q0:512],
                                                 psp[:, 0, q0:512], EXP,
                                                 scale=0.125)
                            nc.scalar.activation(ep[:, 1, q0:512],
                                                 psp[:, 1, q0:512], EXP,
                                                 scale=0.125)
                        if j - 4 * i >= 0:
                            d = j - 4 * i
                            msl = slice(d * P, (d + 1) * P)
                            nc.vector.tensor_mul(ep[:, 0, msl], ep[:, 0, msl],
                                                 mask2[:, 0, :])
                            nc.vector.tensor_mul(ep[:, 1, msl], ep[:, 1, msl],
                                                 mask2[:, 1, :])
                        return ep

                    def emit_norm(b_, qt, psA, off):
                        rinv = nrm_pool.tile([P, 2], f32, tag="rinv",
                                             name=f"rinv{b_}_{qt}")
                        nc.vector.reciprocal(rinv[:, 0:1],
                                             psA[:, off + HD:off + HD + 1])
                        nc.vector.reciprocal(rinv[:, 1:2],
                                             psA[:, off + 129:off + 130])
                        ytn = ytn_pool.tile([P, P], bf16, tag="ytn",
                                            name=f"ytn{b_}_{qt}")
                        nc.vector.tensor_scalar_mul(
                            ytn[:, 0:HD], psA[:, off:off + HD], rinv[:, 0:1])
                        nc.vector.tensor_scalar_mul(
                            ytn[:, HD:P], psA[:, off + HD + 1:off + 129],
                            rinv[:, 1:2])

                        def finish():
                            psT = ps1_pool.tile([P, P], bf16, tag="ps1",
                                                name=f"psT{b_}_{qt}")
                            nc.tensor.transpose(psT[:], ytn[:], ident[:])
                            nc.vector.tensor_copy(
                                yT[:, qt * P:(qt + 1) * P], psT[:])
                        deferred.append(finish)

                    def emit_pv(j, ep):
                        d = max(0, j - 4 * i)
                        for lqt in range(d, 4):
                            qt = 4 * i + lqt
                            psA = pvs[lqt // 2]
                            off = (lqt % 2) * 256
                            esl = slice(lqt * P, (lqt + 1) * P)
                            sp = (j == qt)
                            nc.tensor.matmul(psA[:, off:off + HD + 1],
                                             ep[:, 0, esl],
                                             vaug[:, j, 0:HD + 1],
                                             start=False, stop=sp,
                                             skip_group_check=True)
                            nc.tensor.matmul(psA[:, off + HD + 1:off + 130],
                                             ep[:, 1, esl],
                                             vaug[:, j, HD + 1:2 * HD + 2],
                                             start=False, stop=sp,
                                             skip_group_check=True)
                            if sp:
                                emit_norm(b, qt, psA, off)

                    # software pipeline: S(j+1) issued before PV(j) so the
                    # PE never sits directly behind ACT's exp latency
                    psps = {0: emit_s(0)}
                    eps = {}
                    for j in range(nj):
                        eps[j] = emit_e(j, psps.pop(j))
                        if j + 1 < nj:
                            psps[j + 1] = emit_s(j + 1)
                        emit_pv(j, eps.pop(j))
                        j_done += 1
                        pump(i, j_done, j_total)
                while deferred:
                    deferred.popleft()()
                for q in (late2, late3, fillers):
                    while q:
                        q.popleft()()

            # ---------------- main schedule ----------------
            for u in stage1_units(0, startup=True):
                u()

            # b0: fill with stage1(b1)
            f = collections.deque(stage1_units(1))
            attention(0, f)
            emit_exchange(0)
            load_wp()

            # b1: stage1(b2) + batch-0 proj late (collective X0 ~28us)
            f = collections.deque(stage1_units(2))
            attention(1, f, late3=[proj_unit(0, 0), proj_unit(0, 128)])
            emit_exchange(1)

            # b2: stage1(b3) + batch-1 proj late
            f = collections.deque(stage1_units(3))
            attention(2, f, late3=[proj_unit(1, 0), proj_unit(1, 128)])
            emit_exchange(2)

            # b3: batch-2 proj + row-parallel partials as fillers
            f = collections.deque()
            late2 = [partial_unit(pt) for pt in range(8)]
            late3 = ([partial_unit(pt) for pt in range(8, 12)]
                     + [proj_unit(2, 0), proj_unit(2, 128)])
            attention(3, f, late2=late2, late3=late3)

            # tail: last partial tiles (tokens 1536:2048 of b3)
            for pt in range(12, 16):
                partial_unit(pt)()

    nc.compile()
    return nc


def _prep_inputs(x, w_attn, b_attn, w_proj):
    import ml_dtypes
    bf16 = ml_dtypes.bfloat16

    x = np.asarray(x, dtype=np.float32)
    w_attn = np.asarray(w_attn, dtype=np.float32)
    b_attn = np.asarray(b_attn, dtype=np.float32)
    w_proj = np.asarray(w_proj, dtype=np.float32)

    x_flat = x.reshape(BT, C)
    # xp[tb, p, kt, s] = x_flat[tb*512+s, kt*128+p]
    xp = np.ascontiguousarray(
        x_flat.T.reshape(KT, P, NTB, 512).transpose(2, 1, 0, 3)).astype(bf16)

    wp = np.ascontiguousarray(
        w_proj.reshape(KT, P, C).transpose(1, 0, 2)).astype(bf16)
    in_maps = []
    for c in range(NCORES):
        cols = slice(P * c, P * (c + 1))

        def wslice(off):
            w = w_attn[:, off + P * c: off + P * (c + 1)]   # [1024, 128]
            return np.ascontiguousarray(
                w.reshape(KT, P, P).transpose(1, 0, 2)).astype(bf16)

        in_maps.append({
            "xp": xp,
            "wq": wslice(0),
            "wk": wslice(C),
            "wv": wslice(2 * C),
            "wp": wp,
            "wpr": np.ascontiguousarray(w_proj[cols, :]).astype(bf16),
            "bq": np.ascontiguousarray(b_attn[cols]).reshape(P, 1),
            "bk": np.ascontiguousarray(
                b_attn[C + P * c: C + P * (c + 1)]).reshape(P, 1),
        })
    return in_maps


def kernel(x, w_attn, b_attn, w_proj, b_proj):
    from concourse.bass_utils import run_bass_kernel_spmd

    if "nc" not in _CACHED:
        _CACHED["nc"] = _build_nc()
    nc = _CACHED["nc"]

    in_maps = _prep_inputs(x, w_attn, b_attn, w_proj)
    res = run_bass_kernel_spmd(nc, in_maps, core_ids=list(range(NCORES)))

    w_proj = np.asarray(w_proj, dtype=np.float32)
    b_attn = np.asarray(b_attn, dtype=np.float32)
    y = np.empty((B, T, C), dtype=np.float32)
    for c in range(NCORES):
        r = res.results[c]
        for u in range(3):
            y[u, TPC * c:TPC * (c + 1), :] = r["ypx"][u]
    acc = res.results[0]["ypl"].astype(np.float32).copy()
    for c in range(1, NCORES):
        acc += res.results[c]["ypl"]
    y[3] = acc
    # b_v folded here: y_ref = (attn + b_v) @ w_proj + b_proj
    y += np.asarray(b_proj, dtype=np.float32) + b_attn[2 * C:] @ w_proj
    return y
